# revision 1
# baseline (speedup 1.0000x reference)
"""Trainium2 Bass kernel for nn_BatchConv1d (dynamic per-query conv kernels + banded conv).

Reference computation (per batch b):
    G[i, o]   = (q[b] @ Wk.T + bk)[i, o],  o = c*3 + t   (per-query dynamic kernels)
    bias[i]   = (q[b] @ Wb.T + bb)[i, 0]
    scores[i, j] = sum_{c,t} G[i, c*3+t] * k_pad[b, j+t, c]
    out = scores + bias[:, None] + bias_b

This kernel uses the associativity restructure (2.56x fewer FLOPs):
    N[s, j] = sum_{c,t} Wk[3c+t, s] * k_pad[j+t, c]     (stage 1)
    r[j]    = sum_{c,t} bk[3c+t]    * k_pad[j+t, c]     (bk contribution)
    scores  = q @ N + bias[i] + r[j]                    (stage 2 + fused epilogue)

Sharding: batch data-parallel, 2 batches per core across 8 NeuronCores.
Compute dtype: bf16 matmul inputs, fp32 PSUM accumulation.
"""
import numpy as np

from concourse import bacc, tile, mybir
from concourse.bass_utils import run_bass_kernel_spmd

BF16 = mybir.dt.bfloat16
F32 = mybir.dt.float32
Identity = mybir.ActivationFunctionType.Identity
ADD = mybir.AluOpType.add

B, QL, KL, QS, KS, KW = 16, 1024, 1024, 512, 512, 3
NCORES = 8
B_LOC = B // NCORES      # 2 batches per core
NC_S = QS // 128         # 4 chunks of the s (=QS) contraction dim
NC_C = KS // 128         # 4 chunks of the c (=KS) contraction dim
NI = QL // 128           # 8 i-chunks
NJH = KL // 512          # 2 j-halves

_NC_CACHE = {}


def _build():
    nc = bacc.Bacc("TRN2", target_bir_lowering=False, debug=False)
    q_d = nc.declare_dram_parameter("q", [B_LOC, QL, QS], F32, isOutput=False)
    k_d = nc.declare_dram_parameter("k", [B_LOC, KL, KS], F32, isOutput=False)
    wk_d = nc.declare_dram_parameter("Wk", [KS * KW, QS], F32, isOutput=False)
    bk_d = nc.declare_dram_parameter("bk", [KS * KW], F32, isOutput=False)
    wb_d = nc.declare_dram_parameter("Wb", [1, QS], F32, isOutput=False)
    bb_d = nc.declare_dram_parameter("bb", [1], F32, isOutput=False)
    bias_b_d = nc.declare_dram_parameter("bias_b", [1], F32, isOutput=False)
    id_d = nc.declare_dram_parameter("ident", [128, 128], F32, isOutput=False)
    out_d = nc.declare_dram_parameter("out", [B_LOC, QL, KL], F32, isOutput=True)

    with tile.TileContext(nc) as tc:
        with (
            tc.tile_pool(name="const", bufs=1) as cpool,
            tc.tile_pool(name="wkstage", bufs=1) as wkpool,
            tc.tile_pool(name="io", bufs=2) as iopool,
            tc.tile_pool(name="stage", bufs=1) as spool,
            tc.tile_pool(name="work", bufs=2) as wpool,
            tc.tile_pool(name="outp", bufs=3) as opool,
            tc.tile_pool(name="ps_tp", bufs=3, space="PSUM") as ps_tp,
            tc.tile_pool(name="ps_n", bufs=2, space="PSUM") as ps_n,
            tc.tile_pool(name="ps_aux", bufs=1, space="PSUM") as ps_aux,
            tc.tile_pool(name="ps_s", bufs=2, space="PSUM") as ps_s,
        ):
            # ---- constants ----
            id_sb = cpool.tile([128, 128], F32)
            nc.sync.dma_start(id_sb[:], id_d[:])
            id_bf = cpool.tile([128, 128], BF16)
            nc.gpsimd.dma_start(id_bf[:], id_d[:])
            # Wk as lhsT tiles: wk_sb[t][c][p, s] = Wk[3*(c*128+p)+t, s]
            # staged as f32 via the sync HWDGE ring (ordered after kin), cast on-chip
            wk_sb = [[cpool.tile([128, QS], BF16, tag=f"wk{t}{c}", name=f"wk{t}{c}")
                      for c in range(NC_C)] for t in range(KW)]
            # bk tiles (f32 per-partition scalars): bk_sb[c][p, t] = bk[3*(c*128+p)+t]
            bk_r = bk_d.reshape([KS, KW])
            bk_sb = [cpool.tile([128, KW], F32, tag=f"bk{c}", name=f"bk{c}")
                     for c in range(NC_C)]
            for c in range(NC_C):
                nc.gpsimd.dma_start(bk_sb[c][:], bk_r[c * 128 : (c + 1) * 128, :])
            # Wb^T tiles: [128, 1] per s-chunk
            wb_r = wb_d.reshape([QS, 1])
            wbT_sb = [cpool.tile([128, 1], BF16, tag=f"wb{c}", name=f"wb{c}")
                      for c in range(NC_S)]
            for c in range(NC_S):
                nc.gpsimd.dma_start(wbT_sb[c][:], wb_r[c * 128 : (c + 1) * 128, :])
            ones128 = cpool.tile([128, 128], BF16)
            nc.vector.memset(ones128[:], 1.0)
            # bk broadcast tiles: bkb_sb[t][c][p(c'), m] = bk[3*(c*128+p)+t] for all m
            bkb_sb = [[cpool.tile([128, 128], BF16, tag=f"bkb{t}{c}", name=f"bkb{t}{c}")
                       for c in range(NC_C)] for t in range(KW)]
            for t in range(KW):
                for c in range(NC_C):
                    nc.vector.tensor_scalar_mul(
                        bkb_sb[t][c][:], ones128[:], bk_sb[c][:, t : t + 1]
                    )
            # bb + bias_b scalar, broadcast to all partitions
            bb_sb = cpool.tile([1, 1], F32)
            nc.gpsimd.dma_start(bb_sb[:], bb_d.reshape([1, 1])[:])
            bias_b_sb = cpool.tile([1, 1], F32)
            nc.gpsimd.dma_start(bias_b_sb[:], bias_b_d.reshape([1, 1])[:])
            bbs = cpool.tile([1, 1], F32)
            nc.vector.tensor_add(bbs[:], bb_sb[:], bias_b_sb[:])
            one11 = cpool.tile([1, 1], BF16)
            nc.vector.memset(one11[:], 1.0)

            for b in range(B_LOC):
                # ---- A: load q, k chunks (f32, HWDGE on sync ring: k first, then
                #      Wk (b==0), then q -- FIFO ring order prioritizes k) ----
                qin = [spool.tile([128, QS], F32, tag=f"qin{i}", name=f"qin{i}")
                       for i in range(NI)]
                kin = [spool.tile([128, KS], F32, tag=f"kin{j}", name=f"kin{j}")
                       for j in range(NI)]
                for i in range(NI):
                    nc.sync.dma_start(kin[i][:], k_d[b, i * 128 : (i + 1) * 128, :])
                if b == 0:
                    for c in range(NC_C):
                        for t in range(KW):
                            wkf = wkpool.tile([128, QS], F32, tag=f"wkf{t}{c}",
                                              name=f"wkf{t}{c}")
                            nc.sync.dma_start(
                                wkf[:],
                                wk_d[3 * c * 128 + t : 3 * (c + 1) * 128 : 3, :],
                            )
                            if (t + c) % 2:
                                nc.scalar.activation(wk_sb[t][c][:], wkf[:], Identity)
                            else:
                                nc.vector.tensor_copy(wk_sb[t][c][:], wkf[:])
                for i in range(NI):
                    nc.sync.dma_start(qin[i][:], q_d[b, i * 128 : (i + 1) * 128, :])

                # k chunks to bf16 (casts pipeline behind the chunk DMAs)
                kb = [iopool.tile([128, KS], BF16, tag=f"kb{j}", name=f"kb{j}")
                      for j in range(NI)]
                for j in range(NI):
                    if j % 2 == 0:
                        nc.vector.tensor_copy(kb[j][:], kin[j][:])
                    else:
                        nc.scalar.activation(kb[j][:], kin[j][:], Identity)

                # ---- B: transposes  qT[c]: [128, QL]; kT[c]: [128, KL+2] (bf16) ----
                qT = [wpool.tile([128, QL], BF16, tag=f"qT{c}", name=f"qT{c}")
                      for c in range(NC_S)]
                kT = [wpool.tile([128, KL + 2], BF16, tag=f"kT{c}", name=f"kT{c}")
                      for c in range(NC_C)]
                for c in range(NC_C):
                    nc.vector.memset(kT[c][:, 0:1], 0.0)
                    nc.vector.memset(kT[c][:, KL + 1 : KL + 2], 0.0)
                for g in range(2):
                    for c in range(NC_C):
                        tp = ps_tp.tile([128, 512], BF16, tag="tp")
                        for jj in range(4):
                            j = g * 4 + jj
                            nc.tensor.transpose(
                                tp[:, jj * 128 : (jj + 1) * 128],
                                kb[j][:, c * 128 : (c + 1) * 128],
                                id_bf[:],
                            )
                        if c % 2 == 0:
                            nc.vector.tensor_copy(
                                kT[c][:, 1 + g * 512 : 1 + (g + 1) * 512], tp[:]
                            )
                        else:
                            nc.scalar.activation(
                                kT[c][:, 1 + g * 512 : 1 + (g + 1) * 512], tp[:],
                                Identity,
                            )

                # q chunks to bf16 (emitted after k evacs so they don't block them)
                qb = [iopool.tile([128, QS], BF16, tag=f"qb{i}", name=f"qb{i}")
                      for i in range(NI)]
                for i in range(NI):
                    if i % 2 == 0:
                        nc.vector.tensor_copy(qb[i][:], qin[i][:])
                    else:
                        nc.scalar.activation(qb[i][:], qin[i][:], Identity)

                def q_transpose_group(g, c):
                    tp = ps_tp.tile([128, 512], BF16, tag="tp", name="tp")
                    for ii in range(4):
                        i = g * 4 + ii
                        nc.tensor.transpose(
                            tp[:, ii * 128 : (ii + 1) * 128],
                            qb[i][:, c * 128 : (c + 1) * 128],
                            id_bf[:],
                        )
                    if c % 2 == 0:
                        nc.scalar.activation(
                            qT[c][:, g * 512 : (g + 1) * 512], tp[:], Identity
                        )
                    else:
                        nc.vector.tensor_copy(
                            qT[c][:, g * 512 : (g + 1) * 512], tp[:]
                        )

                # ---- D: R[p, j] = r[j] = sum_{c,t} bk_t[c] * k_pad[j+t, c] ----
                # (only needs kT + consts, so it fills the Wk-load window before stage 1)
                r_sb = wpool.tile([128, KL], F32, tag="rsb")
                for jh in range(NJH):
                    rps = ps_aux.tile([128, 512], F32, tag="aux")
                    first = True
                    for c in range(NC_C):
                        for t in range(KW):
                            nc.tensor.matmul(
                                rps[:],
                                bkb_sb[t][c][:],
                                kT[c][:, jh * 512 + t : jh * 512 + t + 512],
                                start=first,
                                stop=(c == NC_C - 1 and t == KW - 1),
                            )
                            first = False
                    nc.scalar.activation(
                        r_sb[:, jh * 512 : (jh + 1) * 512], rps[:], Identity
                    )

                # ---- C: stage 1  N[s][p, j] = sum_{c,t} Wk_t[c, s] * k_pad[j+t, c] ----
                N = [wpool.tile([128, KL], BF16, tag=f"N{s}", name=f"N{s}")
                     for s in range(NC_S)]
                for s in range(NC_S):
                    for jh in range(NJH):
                        nps = ps_n.tile([128, 512], F32, tag="nps")
                        first = True
                        for c in range(NC_C):
                            for t in range(KW):
                                nc.tensor.matmul(
                                    nps[:],
                                    wk_sb[t][c][:, s * 128 : (s + 1) * 128],
                                    kT[c][:, jh * 512 + t : jh * 512 + t + 512],
                                    start=first,
                                    stop=(c == NC_C - 1 and t == KW - 1),
                                )
                                first = False
                        if (s + jh) % 2 == 0:
                            nc.scalar.activation(
                                N[s][:, jh * 512 : (jh + 1) * 512], nps[:], Identity
                            )
                        else:
                            nc.vector.tensor_copy(
                                N[s][:, jh * 512 : (jh + 1) * 512], nps[:]
                            )
                        q_transpose_group(jh, s)

                # ---- E: bias_row[0, i] = (q @ Wb.T)[i] + bb + bias_b; then to column ----
                bias_row = wpool.tile([1, QL], BF16, tag="brow")
                for ih in range(2):
                    bps = ps_aux.tile([1, 512], F32, tag="aux")
                    for c in range(NC_S):
                        nc.tensor.matmul(
                            bps[:],
                            wbT_sb[c][:],
                            qT[c][:, ih * 512 : (ih + 1) * 512],
                            start=(c == 0),
                            stop=(c == NC_S - 1),
                        )
                    nc.scalar.activation(
                        bias_row[0:1, ih * 512 : (ih + 1) * 512], bps[:], Identity,
                        bias=bbs[:],
                    )
                # row -> column: bias_col[p, i_chunk] = bias_row[0, i_chunk*128 + p]
                bc_ps = ps_aux.tile([128, NI], F32, tag="aux")
                for i in range(NI):
                    nc.tensor.matmul(
                        bc_ps[:, i : i + 1],
                        bias_row[0:1, i * 128 : (i + 1) * 128],
                        one11[:],
                        start=True,
                        stop=True,
                    )
                bias_col = wpool.tile([128, NI], F32, tag="bcol")
                nc.vector.tensor_copy(bias_col[:], bc_ps[:])

                # ---- F: stage 2 + fused epilogue ----
                for i in range(NI):
                    out_sb = opool.tile([128, KL], F32, tag="osb")
                    for jh in range(NJH):
                        sps = ps_s.tile([128, 512], F32, tag="sps")
                        for c in range(NC_S):
                            nc.tensor.matmul(
                                sps[:],
                                qT[c][:, i * 128 : (i + 1) * 128],
                                N[c][:, jh * 512 : (jh + 1) * 512],
                                start=(c == 0),
                                stop=(c == NC_S - 1),
                            )
                        # out = (sps + bias_col[i]) + r
                        nc.vector.scalar_tensor_tensor(
                            out_sb[:, jh * 512 : (jh + 1) * 512],
                            sps[:],
                            bias_col[:, i : i + 1],
                            r_sb[:, jh * 512 : (jh + 1) * 512],
                            ADD,
                            ADD,
                        )
                        nc.scalar.dma_start(
                            out_d[b, i * 128 : (i + 1) * 128,
                                  jh * 512 : (jh + 1) * 512],
                            out_sb[:, jh * 512 : (jh + 1) * 512],
                        )
    nc.finalize()
    return nc


def _get_nc():
    if "nc" not in _NC_CACHE:
        _NC_CACHE["nc"] = _build()
    return _NC_CACHE["nc"]


def kernel(q, k, Wk, bk, Wb, bb, bias_b):
    nc = _get_nc()
    ident = np.eye(128, dtype=np.float32)
    in_maps = []
    for core in range(NCORES):
        lo, hi = core * B_LOC, (core + 1) * B_LOC
        in_maps.append({
            "q": np.ascontiguousarray(np.asarray(q, dtype=np.float32)[lo:hi]),
            "k": np.ascontiguousarray(np.asarray(k, dtype=np.float32)[lo:hi]),
            "Wk": np.asarray(Wk, dtype=np.float32),
            "bk": np.asarray(bk, dtype=np.float32),
            "Wb": np.asarray(Wb, dtype=np.float32),
            "bb": np.asarray(bb, dtype=np.float32),
            "bias_b": np.asarray(bias_b, dtype=np.float32),
            "ident": ident,
        })
    res = run_bass_kernel_spmd(nc, in_maps, list(range(NCORES)))
    return np.concatenate([res.results[c]["out"] for c in range(NCORES)], axis=0)



# revision 3
# speedup vs baseline: 1.1293x; 1.1293x over previous
"""Trainium2 Bass kernel for nn_BatchConv1d (dynamic per-query conv kernels + banded conv).

Reference computation (per batch b):
    G[i, o]   = (q[b] @ Wk.T + bk)[i, o],  o = c*3 + t   (per-query dynamic kernels)
    bias[i]   = (q[b] @ Wb.T + bb)[i, 0]
    scores[i, j] = sum_{c,t} G[i, c*3+t] * k_pad[b, j+t, c]
    out = scores + bias[:, None] + bias_b

Associativity restructure (2.56x fewer FLOPs than the direct form):
    N[s, j] = sum_{c,t} Wk[3c+t, s] * k_pad[j+t, c]     (stage 1)
    r[j]    = sum_{c,t} bk[3c+t]    * k_pad[j+t, c]     (bk contribution)
    scores  = q @ N' + r'[j],   N'[s,j] = N[s,j] + Wb[0,s]   (Wb folded into N,
              so q @ N' automatically adds the per-query bias; bb + bias_b are
              folded into r' at its PSUM evacuation)

All transposes / dtype casts / weight re-packing are done host-side (numpy),
so the device only runs the three matmul stages plus PSUM evacuations:
  - qT  [512, 1024] bf16  (per batch)      - stage-2 stationary
  - kT  [512, 1026] bf16  (per batch, with zero halo) - stage-1/r moving
  - wk  [4, 128, 1536] bf16 re-packed Wk   - stage-1 stationary
  - bkb [128, 3072] bf16 broadcast bk      - r stationary
  - cons [128, 8] f32: cols 0..3 = Wb^T per s-chunk, col 4 = bb + bias_b
Output is written bf16 and upcast to f32 on host.

Sharding: batch data-parallel, 2 batches per core across 8 NeuronCores.
"""
import numpy as np
import ml_dtypes

from concourse import bacc, tile, mybir
from concourse.bass_utils import run_bass_kernel_spmd

BF16 = mybir.dt.bfloat16
F32 = mybir.dt.float32
BF = ml_dtypes.bfloat16
Identity = mybir.ActivationFunctionType.Identity
ADD = mybir.AluOpType.add

B, QL, KL, QS, KS, KW = 16, 1024, 1024, 512, 512, 3
NCORES = 8
B_LOC = B // NCORES      # 2 batches per core
NC_S = QS // 128         # 4 chunks of the s (=QS) contraction dim
NC_C = KS // 128         # 4 chunks of the c (=KS) contraction dim
NI = QL // 128           # 8 i-chunks
NJH = KL // 512          # 2 j-halves

_NC_CACHE = {}


def _build():
    nc = bacc.Bacc("TRN2", target_bir_lowering=False, debug=False)
    qt_d = nc.declare_dram_parameter("qT", [B_LOC, QS, QL], BF16, isOutput=False)
    kt_d = nc.declare_dram_parameter("kT", [B_LOC, KS, KL + 2], BF16, isOutput=False)
    wk_d = nc.declare_dram_parameter("wk", [NC_C, 128, KW * QS], BF16, isOutput=False)
    bkb_d = nc.declare_dram_parameter("bkb", [128, KW * NC_C * 128], BF16, isOutput=False)
    cons_d = nc.declare_dram_parameter("cons", [128, 8], F32, isOutput=False)
    out_d = nc.declare_dram_parameter("out", [B_LOC, QL, KL], BF16, isOutput=True)

    with tile.TileContext(nc) as tc:
        with (
            tc.tile_pool(name="const", bufs=1) as cpool,
            tc.tile_pool(name="kq", bufs=2) as kqpool,
            tc.tile_pool(name="nr", bufs=2) as npool,
            tc.tile_pool(name="outp", bufs=3) as opool,
            tc.tile_pool(name="ps_n", bufs=2, space="PSUM") as ps_n,
            tc.tile_pool(name="ps_r", bufs=2, space="PSUM") as ps_r,
            tc.tile_pool(name="ps_s", bufs=3, space="PSUM") as ps_s,
        ):
            # ---- constants (gpsimd ring; wide DMAs to amortize descriptors) ----
            cons_sb = cpool.tile([128, 8], F32, tag="cons", name="cons")
            nc.gpsimd.dma_start(cons_sb[:], cons_d[:])
            # wk_sb[c][p, t*512 + s] = Wk[3*(c*128+p) + t, s]; one DMA per c-chunk
            wk_sb = [cpool.tile([128, KW * QS], BF16, tag=f"wk{c}", name=f"wk{c}")
                     for c in range(NC_C)]
            for c in range(NC_C):
                nc.gpsimd.dma_start(wk_sb[c][:], wk_d[c, :, :])
            # bkb_sb[p, (t*4+c)*128 + m] = bk[3*(c*128+p) + t]  (m broadcast)
            bkb_sb = cpool.tile([128, KW * NC_C * 128], BF16, tag="bkb", name="bkb")
            nc.gpsimd.dma_start(bkb_sb[:], bkb_d[:])

            for b in range(B_LOC):
                # ---- loads (sync ring): kT first (stage 1), then qT (stage 2)
                kT = [kqpool.tile([128, KL + 2], BF16, tag=f"kT{c}", name=f"kT{c}")
                      for c in range(NC_C)]
                for c in range(NC_C):
                    nc.sync.dma_start(kT[c][:], kt_d[b, c * 128 : (c + 1) * 128, :])
                qT = [kqpool.tile([128, QL], BF16, tag=f"qT{c}", name=f"qT{c}")
                      for c in range(NC_S)]
                for c in range(NC_S):
                    nc.sync.dma_start(qT[c][:], qt_d[b, c * 128 : (c + 1) * 128, :])

                # ---- stage 1: N'[s][p, j] = sum_{c,t} Wk[3c+t, s]*k_pad[j+t, c] + Wb[s]
                N = [npool.tile([128, KL], BF16, tag=f"N{s}", name=f"N{s}")
                     for s in range(NC_S)]
                for jh in range(NJH):
                    for s in range(NC_S):
                        nps = ps_n.tile([128, 512], F32, tag="nps")
                        first = True
                        for c in range(NC_C):
                            for t in range(KW):
                                nc.tensor.matmul(
                                    nps[:],
                                    wk_sb[c][:, t * QS + s * 128 : t * QS + (s + 1) * 128],
                                    kT[c][:, jh * 512 + t : jh * 512 + t + 512],
                                    start=first,
                                    stop=(c == NC_C - 1 and t == KW - 1),
                                )
                                first = False
                        nc.scalar.activation(
                            N[s][:, jh * 512 : (jh + 1) * 512], nps[:], Identity,
                            bias=cons_sb[:, s : s + 1],
                        )

                # ---- r'[p, j] = sum_{c,t} bk[3c+t]*k_pad[j+t, c] + bb + bias_b
                #      (all partitions identical via broadcast bk tiles)
                r_sb = npool.tile([128, KL], F32, tag="rsb", name="rsb")
                for jh in range(NJH):
                    rps = ps_r.tile([128, 512], F32, tag="rps")
                    first = True
                    for c in range(NC_C):
                        for t in range(KW):
                            nc.tensor.matmul(
                                rps[:],
                                bkb_sb[:, (t * NC_C + c) * 128 : (t * NC_C + c + 1) * 128],
                                kT[c][:, jh * 512 + t : jh * 512 + t + 512],
                                start=first,
                                stop=(c == NC_C - 1 and t == KW - 1),
                            )
                            first = False
                    nc.scalar.activation(
                        r_sb[:, jh * 512 : (jh + 1) * 512], rps[:], Identity,
                        bias=cons_sb[:, 4:5],
                    )

                # ---- stage 2 + fused epilogue: out = qT.T @ N' + r'
                for i in range(NI):
                    out_sb = opool.tile([128, KL], BF16, tag="osb")
                    for jh in range(NJH):
                        sps = ps_s.tile([128, 512], F32, tag="sps")
                        for c in range(NC_S):
                            nc.tensor.matmul(
                                sps[:],
                                qT[c][:, i * 128 : (i + 1) * 128],
                                N[c][:, jh * 512 : (jh + 1) * 512],
                                start=(c == 0),
                                stop=(c == NC_S - 1),
                            )
                        nc.vector.tensor_tensor(
                            out_sb[:, jh * 512 : (jh + 1) * 512],
                            sps[:],
                            r_sb[:, jh * 512 : (jh + 1) * 512],
                            ADD,
                        )
                    nc.gpsimd.dma_start(
                        out_d[b, i * 128 : (i + 1) * 128, :], out_sb[:]
                    )
    nc.finalize()
    return nc


def _get_nc():
    if "nc" not in _NC_CACHE:
        _NC_CACHE["nc"] = _build()
    return _NC_CACHE["nc"]


def _prep_inputs(q, k, Wk, bk, Wb, bb, bias_b):
    q = np.asarray(q, np.float32)
    k = np.asarray(k, np.float32)
    Wk = np.asarray(Wk, np.float32)
    bk = np.asarray(bk, np.float32)
    Wb = np.asarray(Wb, np.float32)
    bb = np.asarray(bb, np.float32)
    bias_b = np.asarray(bias_b, np.float32)

    qT = q.transpose(0, 2, 1).astype(BF)                    # [B, QS, QL]
    kT = np.zeros((B, KS, KL + 2), dtype=BF)
    kT[:, :, 1 : KL + 1] = k.transpose(0, 2, 1).astype(BF)  # zero halo cols 0, KL+1
    # wk[c][p, t*QS + s] = Wk[3*(c*128+p) + t, s]
    wk = np.ascontiguousarray(Wk.reshape(NC_C, 128, KW, QS)).reshape(
        NC_C, 128, KW * QS).astype(BF)
    # bkb[p, (t*4+c)*128 + m] = bk[3*(c*128+p) + t]
    bkb = np.ascontiguousarray(
        np.broadcast_to(
            bk.reshape(NC_C, 128, KW).transpose(1, 2, 0)[:, :, :, None],
            (128, KW, NC_C, 128),
        )
    ).reshape(128, KW * NC_C * 128).astype(BF)
    cons = np.zeros((128, 8), np.float32)
    cons[:, 0:NC_S] = Wb.reshape(NC_S, 128).T               # Wb^T per s-chunk
    cons[:, 4] = bb[0] + bias_b[0]

    in_maps = []
    for core in range(NCORES):
        lo = core * B_LOC
        in_maps.append({
            "qT": np.ascontiguousarray(qT[lo : lo + B_LOC]),
            "kT": kT[lo : lo + B_LOC],
            "wk": wk,
            "bkb": bkb,
            "cons": cons,
        })
    return in_maps


def kernel(q, k, Wk, bk, Wb, bb, bias_b):
    nc = _get_nc()
    in_maps = _prep_inputs(q, k, Wk, bk, Wb, bb, bias_b)
    res = run_bass_kernel_spmd(nc, in_maps, list(range(NCORES)))
    out = np.concatenate([np.asarray(res.results[c]["out"]) for c in range(NCORES)],
                         axis=0)
    return out.astype(np.float32)


# revision 9
# speedup vs baseline: 1.1311x; 1.0015x over previous
"""Trainium2 Bass kernel for nn_BatchConv1d (dynamic per-query conv kernels + banded conv).

Reference computation (per batch b):
    G[i, o]   = (q[b] @ Wk.T + bk)[i, o],  o = c*3 + t   (per-query dynamic kernels)
    bias[i]   = (q[b] @ Wb.T + bb)[i, 0]
    scores[i, j] = sum_{c,t} G[i, c*3+t] * k_pad[b, j+t, c]
    out = scores + bias[:, None] + bias_b

Associativity restructure (2.56x fewer FLOPs than the direct form):
    N[s, j] = sum_{c,t} Wk[3c+t, s] * k_pad[j+t, c]     (stage 1)
    P[t,jj] = sum_c    bk[3c+t]     * k_pad[jj, c]      (bk contribution, 3 rows)
    r[j]    = sum_t P[t, j+t] + bb + bias_b             (rank-1 shift-sum on PE)
    scores  = q @ N' + r'[j],   N'[s,j] = N[s,j] + Wb[0,s]   (Wb folded into N,
              so q @ N' automatically adds the per-query bias)

All transposes / dtype casts / weight re-packing are done host-side (numpy),
so the device only runs the matmul stages plus PSUM evacuations:
  - qT  [512, 1024] bf16  (per batch)      - stage-2 stationary
  - kT  [512, 1026] bf16  (per batch, with zero halo) - stage-1/P moving
  - wk  [4, 128, 1536] bf16 re-packed Wk   - stage-1 stationary
  - bkp [128, 12] bf16: bkp[p, c*3+t] = bk[3*(c*128+p)+t] - P stationary
  - cons [128, 8] f32: cols 0..3 = Wb^T per s-chunk, col 4 = bb + bias_b
Output is written bf16 and upcast to f32 on host.

Sharding: batch data-parallel, 2 batches per core across 8 NeuronCores.
"""
import numpy as np
import ml_dtypes

from concourse import bacc, tile, mybir
from concourse.bass_utils import run_bass_kernel_spmd

BF16 = mybir.dt.bfloat16
F32 = mybir.dt.float32
BF = ml_dtypes.bfloat16
Identity = mybir.ActivationFunctionType.Identity
ADD = mybir.AluOpType.add

B, QL, KL, QS, KS, KW = 16, 1024, 1024, 512, 512, 3
NCORES = 8
B_LOC = B // NCORES      # 2 batches per core
NC_S = QS // 128         # 4 chunks of the s (=QS) contraction dim
NC_C = KS // 128         # 4 chunks of the c (=KS) contraction dim
NI = QL // 128           # 8 i-chunks
NJH = KL // 512          # 2 j-halves
KH = (KL + 2) // 2       # 514: kT half-width (with halo)

_NC_CACHE = {}


def _build():
    nc = bacc.Bacc("TRN2", target_bir_lowering=False, debug=False)
    qt_d = nc.declare_dram_parameter("qT", [B_LOC, QS, QL], BF16, isOutput=False)
    kt_d = nc.declare_dram_parameter("kT", [B_LOC, KS, KL + 2], BF16, isOutput=False)
    wk_d = nc.declare_dram_parameter("wk", [NC_C, 128, KW * QS], BF16, isOutput=False)
    bkb_d = nc.declare_dram_parameter("bkb", [128, KW * NC_C * 128], BF16, isOutput=False)
    cons_d = nc.declare_dram_parameter("cons", [128, 8], F32, isOutput=False)
    out_d = nc.declare_dram_parameter("out", [B_LOC, QL, KL], BF16, isOutput=True)

    with tile.TileContext(nc) as tc:
        with (
            tc.tile_pool(name="const", bufs=1) as cpool,
            tc.tile_pool(name="kq", bufs=2) as kqpool,
            tc.tile_pool(name="nr", bufs=2) as npool,
            tc.tile_pool(name="outp", bufs=3) as opool,
            tc.tile_pool(name="ps_n", bufs=2, space="PSUM") as ps_n,
            tc.tile_pool(name="ps_r", bufs=2, space="PSUM") as ps_r,
            tc.tile_pool(name="ps_s", bufs=3, space="PSUM") as ps_s,
        ):
            # ---- constants (gpsimd ring) ----
            cons_sb = cpool.tile([128, 8], F32, tag="cons", name="cons")
            nc.gpsimd.dma_start(cons_sb[:], cons_d[:])
            bkb_sb = cpool.tile([128, KW * NC_C * 128], BF16, tag="bkb", name="bkb")
            nc.gpsimd.dma_start(bkb_sb[:], bkb_d[:])
            # wk_sb[c][p, t*512 + s] = Wk[3*(c*128+p) + t, s]; per-(c,t) DMAs so
            # the first stage-1 matmuls can start while later slices stream in
            wk_sb = [cpool.tile([128, KW * QS], BF16, tag=f"wk{c}", name=f"wk{c}")
                     for c in range(NC_C)]
            for c in range(NC_C):
                for t in range(KW):
                    nc.gpsimd.dma_start(
                        wk_sb[c][:, t * QS : (t + 1) * QS],
                        wk_d[c, :, t * QS : (t + 1) * QS],
                    )

            for b in range(B_LOC):
                # ---- loads (sync ring): kT halves first (stage 1), then qT
                kT = [kqpool.tile([128, KL + 2], BF16, tag=f"kT{c}", name=f"kT{c}")
                      for c in range(NC_C)]
                for h in range(2):
                    for c in range(NC_C):
                        nc.sync.dma_start(
                            kT[c][:, h * KH : (h + 1) * KH],
                            kt_d[b, c * 128 : (c + 1) * 128, h * KH : (h + 1) * KH],
                        )
                qT = [kqpool.tile([128, QL], BF16, tag=f"qT{c}", name=f"qT{c}")
                      for c in range(NC_S)]
                for c in range(NC_S):
                    nc.sync.dma_start(qT[c][:], qt_d[b, c * 128 : (c + 1) * 128, :])

                # ---- stage 1: N'[s][p, j] = sum_{c,t} Wk[3c+t, s]*k_pad[j+t, c] + Wb[s]
                N = [npool.tile([128, KL], BF16, tag=f"N{s}", name=f"N{s}")
                     for s in range(NC_S)]
                for jh in range(NJH):
                    for s in range(NC_S):
                        nps = ps_n.tile([128, 512], F32, tag="nps")
                        first = True
                        for c in range(NC_C):
                            for t in range(KW):
                                nc.tensor.matmul(
                                    nps[:],
                                    wk_sb[c][:, t * QS + s * 128 : t * QS + (s + 1) * 128],
                                    kT[c][:, jh * 512 + t : jh * 512 + t + 512],
                                    start=first,
                                    stop=(c == NC_C - 1 and t == KW - 1),
                                )
                                first = False
                        nc.scalar.activation(
                            N[s][:, jh * 512 : (jh + 1) * 512], nps[:], Identity,
                            bias=cons_sb[:, s : s + 1],
                        )

                # ---- r'[p, j] = sum_{c,t} bk[3c+t]*k_pad[j+t, c] + bb + bias_b
                #      (all partitions identical via broadcast bk tiles)
                r_sb = npool.tile([128, KL], F32, tag="rsb", name="rsb")
                for jh in range(NJH):
                    rps = ps_r.tile([128, 512], F32, tag="rps")
                    first = True
                    for c in range(NC_C):
                        for t in range(KW):
                            nc.tensor.matmul(
                                rps[:],
                                bkb_sb[:, (t * NC_C + c) * 128 : (t * NC_C + c + 1) * 128],
                                kT[c][:, jh * 512 + t : jh * 512 + t + 512],
                                start=first,
                                stop=(c == NC_C - 1 and t == KW - 1),
                            )
                            first = False
                    nc.scalar.activation(
                        r_sb[:, jh * 512 : (jh + 1) * 512], rps[:], Identity,
                        bias=cons_sb[:, 4:5],
                    )

                # ---- stage 2 + fused epilogue: out = qT.T @ N' + r'
                for i in range(NI):
                    out_sb = opool.tile([128, KL], BF16, tag="osb")
                    for jh in range(NJH):
                        sps = ps_s.tile([128, 512], F32, tag="sps")
                        for c in range(NC_S):
                            nc.tensor.matmul(
                                sps[:],
                                qT[c][:, i * 128 : (i + 1) * 128],
                                N[c][:, jh * 512 : (jh + 1) * 512],
                                start=(c == 0),
                                stop=(c == NC_S - 1),
                            )
                        nc.vector.tensor_tensor(
                            out_sb[:, jh * 512 : (jh + 1) * 512],
                            sps[:],
                            r_sb[:, jh * 512 : (jh + 1) * 512],
                            ADD,
                        )
                    eng = nc.gpsimd if i % 2 == 0 else nc.scalar
                    eng.dma_start(out_d[b, i * 128 : (i + 1) * 128, :], out_sb[:])
    nc.finalize()
    return nc


def _get_nc():
    if "nc" not in _NC_CACHE:
        _NC_CACHE["nc"] = _build()
    return _NC_CACHE["nc"]


def _prep_inputs(q, k, Wk, bk, Wb, bb, bias_b):
    q = np.asarray(q, np.float32)
    k = np.asarray(k, np.float32)
    Wk = np.asarray(Wk, np.float32)
    bk = np.asarray(bk, np.float32)
    Wb = np.asarray(Wb, np.float32)
    bb = np.asarray(bb, np.float32)
    bias_b = np.asarray(bias_b, np.float32)

    qT = q.transpose(0, 2, 1).astype(BF)                    # [B, QS, QL]
    kT = np.zeros((B, KS, KL + 2), dtype=BF)
    kT[:, :, 1 : KL + 1] = k.transpose(0, 2, 1).astype(BF)  # zero halo cols 0, KL+1
    # wk[c][p, t*QS + s] = Wk[3*(c*128+p) + t, s]
    wk = np.ascontiguousarray(Wk.reshape(NC_C, 128, KW, QS)).reshape(
        NC_C, 128, KW * QS).astype(BF)
    # bkb[p, (t*4+c)*128 + m] = bk[3*(c*128+p) + t]
    bkb = np.ascontiguousarray(
        np.broadcast_to(
            bk.reshape(NC_C, 128, KW).transpose(1, 2, 0)[:, :, :, None],
            (128, KW, NC_C, 128),
        )
    ).reshape(128, KW * NC_C * 128).astype(BF)
    cons = np.zeros((128, 8), np.float32)
    cons[:, 0:NC_S] = Wb.reshape(NC_S, 128).T               # Wb^T per s-chunk
    cons[:, 4] = bb[0] + bias_b[0]

    in_maps = []
    for core in range(NCORES):
        lo = core * B_LOC
        in_maps.append({
            "qT": np.ascontiguousarray(qT[lo : lo + B_LOC]),
            "kT": kT[lo : lo + B_LOC],
            "wk": wk,
            "bkb": bkb,
            "cons": cons,
        })
    return in_maps


def kernel(q, k, Wk, bk, Wb, bb, bias_b):
    nc = _get_nc()
    in_maps = _prep_inputs(q, k, Wk, bk, Wb, bb, bias_b)
    res = run_bass_kernel_spmd(nc, in_maps, list(range(NCORES)))
    out = np.concatenate([np.asarray(res.results[c]["out"]) for c in range(NCORES)],
                         axis=0)
    return out.astype(np.float32)


# revision 22
# speedup vs baseline: 1.1842x; 1.0470x over previous
"""Trainium2 Bass kernel for nn_BatchConv1d (dynamic per-query conv kernels + banded conv).

Reference computation (per batch b):
    G[i, o]   = (q[b] @ Wk.T + bk)[i, o],  o = c*3 + t   (per-query dynamic kernels)
    bias[i]   = (q[b] @ Wb.T + bb)[i, 0]
    scores[i, j] = sum_{c,t} G[i, c*3+t] * k_pad[b, j+t, c]
    out = scores + bias[:, None] + bias_b

Associativity restructure (2.56x fewer FLOPs than the direct form):
    N[s, j] = sum_{c,t} Wk[3c+t, s] * k_pad[j+t, c]     (stage 1)
    P[t,jj] = sum_c    bk[3c+t]     * k_pad[jj, c]      (bk contribution, 3 rows)
    r[j]    = sum_t P[t, j+t] + bb + bias_b             (rank-1 shift-sum on PE)
    scores  = q @ N' + r'[j],   N'[s,j] = N[s,j] + Wb[0,s]   (Wb folded into N,
              so q @ N' automatically adds the per-query bias)

All transposes / dtype casts / weight re-packing are done host-side (numpy),
so the device only runs the matmul stages plus PSUM evacuations:
  - qT  [512, 1024] bf16  (per batch)      - stage-2 stationary
  - kT  [512, 1026] bf16  (per batch, with zero halo) - stage-1/P moving
  - wk  [4, 128, 1536] bf16 re-packed Wk   - stage-1 stationary
  - bkp [128, 12] bf16: bkp[p, c*3+t] = bk[3*(c*128+p)+t] - P stationary
  - cons [128, 8] f32: cols 0..3 = Wb^T per s-chunk, col 4 = bb + bias_b
Output is written bf16 and upcast to f32 on host.

Sharding: batch data-parallel, 2 batches per core across 8 NeuronCores.
"""
import numpy as np
import ml_dtypes

from concourse import bacc, tile, mybir
from concourse.bass_utils import run_bass_kernel_spmd

BF16 = mybir.dt.bfloat16
F32 = mybir.dt.float32
BF = ml_dtypes.bfloat16
Identity = mybir.ActivationFunctionType.Identity
ADD = mybir.AluOpType.add

B, QL, KL, QS, KS, KW = 16, 1024, 1024, 512, 512, 3
NCORES = 8
B_LOC = B // NCORES      # 2 batches per core
NC_S = QS // 128         # 4 chunks of the s (=QS) contraction dim
NC_C = KS // 128         # 4 chunks of the c (=KS) contraction dim
NI = QL // 128           # 8 i-chunks
NJH = KL // 512          # 2 j-halves
KH = (KL + 2) // 2       # 514: kT half-width (with halo)

_NC_CACHE = {}


def _build():
    nc = bacc.Bacc("TRN2", target_bir_lowering=False, debug=False)
    qt_d = nc.declare_dram_parameter("qT", [B_LOC, QS, QL], BF16, isOutput=False)
    kt_d = nc.declare_dram_parameter("kT", [B_LOC, KS, KL + 2], BF16, isOutput=False)
    wk_d = nc.declare_dram_parameter("wk", [NC_C, 128, KW * QS], BF16, isOutput=False)
    bkp_d = nc.declare_dram_parameter("bkp", [128, NC_C * KW], BF16, isOutput=False)
    cons_d = nc.declare_dram_parameter("cons", [128, 8], F32, isOutput=False)
    out_d = nc.declare_dram_parameter("out", [B_LOC, QL, KL], BF16, isOutput=True)

    with tile.TileContext(nc) as tc:
        with (
            tc.tile_pool(name="const", bufs=1) as cpool,
            tc.tile_pool(name="kq", bufs=2) as kqpool,
            tc.tile_pool(name="nr", bufs=2) as npool,
            tc.tile_pool(name="outp", bufs=3) as opool,
            tc.tile_pool(name="ps_n", bufs=2, space="PSUM") as ps_n,
            tc.tile_pool(name="ps_p", bufs=1, space="PSUM") as ps_p,
            tc.tile_pool(name="ps_r", bufs=1, space="PSUM") as ps_r,
            tc.tile_pool(name="ps_s", bufs=4, space="PSUM") as ps_s,
        ):
            # ---- constants (gpsimd ring) ----
            cons_sb = cpool.tile([128, 8], F32, tag="cons", name="cons")
            nc.gpsimd.dma_start(cons_sb[:], cons_d[:])
            bkp_sb = cpool.tile([128, NC_C * KW], BF16, tag="bkp", name="bkp")
            nc.gpsimd.dma_start(bkp_sb[:], bkp_d[:])
            # all-ones stationary for the rank-1 broadcast matmuls of r
            ones_sb = cpool.tile([1, 128], BF16, tag="ones", name="ones")
            nc.vector.memset(ones_sb[:], 1.0)
            # wk_sb[c][p, t*512 + s] = Wk[3*(c*128+p) + t, s]; per-(c,t) DMAs so
            # the first stage-1 matmuls can start while later slices stream in
            wk_sb = [cpool.tile([128, KW * QS], BF16, tag=f"wk{c}", name=f"wk{c}")
                     for c in range(NC_C)]
            for c in range(NC_C):
                for t in range(KW):
                    nc.gpsimd.dma_start(
                        wk_sb[c][:, t * QS : (t + 1) * QS],
                        wk_d[c, :, t * QS : (t + 1) * QS],
                    )

            for b in range(B_LOC):
                # ---- loads (sync ring): kT halves first (stage 1), then qT
                kT = [kqpool.tile([128, KL + 2], BF16, tag=f"kT{c}", name=f"kT{c}")
                      for c in range(NC_C)]
                for h in range(2):
                    for c in range(NC_C):
                        nc.sync.dma_start(
                            kT[c][:, h * KH : (h + 1) * KH],
                            kt_d[b, c * 128 : (c + 1) * 128, h * KH : (h + 1) * KH],
                        )
                qT = [kqpool.tile([128, QL], BF16, tag=f"qT{c}", name=f"qT{c}")
                      for c in range(NC_S)]
                for c in range(NC_S):
                    nc.sync.dma_start(qT[c][:], qt_d[b, c * 128 : (c + 1) * 128, :])

                # ---- P[t, jj] = sum_c bk[3c+t] * k_pad[jj, c]   ([3, 1026])
                #      Rows 1,2 are copied to their own tiles so every rank-1
                #      matmul operand sits at base partition 0 (PE rule).
                #      Chunks A+B only need the low kT halves -> emitted first.
                P_sb = npool.tile([3, KL + 2], BF16, tag="psb", name="psb")
                P1T = npool.tile([1, KL + 2], BF16, tag="p1t", name="p1t")
                P2T = npool.tile([1, KL + 2], BF16, tag="p2t", name="p2t")

                def p_chunk(off, sz):
                    pps = ps_p.tile([3, 512], F32, tag="pps")
                    for c in range(NC_C):
                        nc.tensor.matmul(
                            pps[0:3, 0:sz],
                            bkp_sb[:, c * KW : (c + 1) * KW],
                            kT[c][:, off : off + sz],
                            start=(c == 0),
                            stop=(c == NC_C - 1),
                        )
                    nc.scalar.activation(
                        P_sb[0:3, off : off + sz], pps[0:3, 0:sz], Identity
                    )

                p_chunk(0, 512)
                p_chunk(512, 2)

                # ---- stage 1: N'[s][p, j] = sum_{c,t} Wk[3c+t, s]*k_pad[j+t, c] + Wb[s]
                N = [npool.tile([128, KL], BF16, tag=f"N{s}", name=f"N{s}")
                     for s in range(NC_S)]
                for jh in range(NJH):
                    if jh == 1:
                        p_chunk(514, 510)
                        p_chunk(1024, 2)
                        nc.sync.dma_start(P1T[0:1, :], P_sb[1:2, :])
                        nc.sync.dma_start(P2T[0:1, :], P_sb[2:3, :])
                    for s in range(NC_S):
                        nps = ps_n.tile([128, 512], F32, tag="nps")
                        first = True
                        for c in range(NC_C):
                            for t in range(KW):
                                nc.tensor.matmul(
                                    nps[:],
                                    wk_sb[c][:, t * QS + s * 128 : t * QS + (s + 1) * 128],
                                    kT[c][:, jh * 512 + t : jh * 512 + t + 512],
                                    start=first,
                                    stop=(c == NC_C - 1 and t == KW - 1),
                                )
                                first = False
                        nc.scalar.activation(
                            N[s][:, jh * 512 : (jh + 1) * 512], nps[:], Identity,
                            bias=cons_sb[:, s : s + 1],
                        )

                # ---- r'[p, j] = sum_t P[t, j+t] + bb + bias_b  (rank-1 shift-sum,
                #      all partitions identical)
                r_sb = npool.tile([128, KL], F32, tag="rsb", name="rsb")
                for jh in range(NJH):
                    rps = ps_r.tile([128, 512], F32, tag="rps")
                    for t, row in ((0, P_sb), (1, P1T), (2, P2T)):
                        nc.tensor.matmul(
                            rps[:],
                            ones_sb[:],
                            row[0:1, jh * 512 + t : jh * 512 + t + 512],
                            start=(t == 0),
                            stop=(t == KW - 1),
                        )
                    nc.scalar.activation(
                        r_sb[:, jh * 512 : (jh + 1) * 512], rps[:], Identity,
                        bias=cons_sb[:, 4:5],
                    )

                # ---- stage 2 + fused epilogue: out = qT.T @ N' + r'
                for i in range(NI):
                    out_sb = opool.tile([128, KL], BF16, tag="osb")
                    for jh in range(NJH):
                        sps = ps_s.tile([128, 512], F32, tag="sps")
                        for c in range(NC_S):
                            nc.tensor.matmul(
                                sps[:],
                                qT[c][:, i * 128 : (i + 1) * 128],
                                N[c][:, jh * 512 : (jh + 1) * 512],
                                start=(c == 0),
                                stop=(c == NC_S - 1),
                            )
                        nc.vector.tensor_tensor(
                            out_sb[:, jh * 512 : (jh + 1) * 512],
                            sps[:],
                            r_sb[:, jh * 512 : (jh + 1) * 512],
                            ADD,
                        )
                    eng = nc.gpsimd if i % 2 == 0 else nc.scalar
                    eng.dma_start(out_d[b, i * 128 : (i + 1) * 128, :], out_sb[:])
    nc.finalize()
    return nc


def _get_nc():
    if "nc" not in _NC_CACHE:
        _NC_CACHE["nc"] = _build()
    return _NC_CACHE["nc"]


def _prep_inputs(q, k, Wk, bk, Wb, bb, bias_b):
    q = np.asarray(q, np.float32)
    k = np.asarray(k, np.float32)
    Wk = np.asarray(Wk, np.float32)
    bk = np.asarray(bk, np.float32)
    Wb = np.asarray(Wb, np.float32)
    bb = np.asarray(bb, np.float32)
    bias_b = np.asarray(bias_b, np.float32)

    qT = q.transpose(0, 2, 1).astype(BF)                    # [B, QS, QL]
    kT = np.zeros((B, KS, KL + 2), dtype=BF)
    kT[:, :, 1 : KL + 1] = k.transpose(0, 2, 1).astype(BF)  # zero halo cols 0, KL+1
    # wk[c][p, t*QS + s] = Wk[3*(c*128+p) + t, s]
    wk = np.ascontiguousarray(Wk.reshape(NC_C, 128, KW, QS)).reshape(
        NC_C, 128, KW * QS).astype(BF)
    # bkp[p, c*3 + t] = bk[3*(c*128+p) + t]
    bkp = np.ascontiguousarray(
        bk.reshape(NC_C, 128, KW).transpose(1, 0, 2)).reshape(
        128, NC_C * KW).astype(BF)
    cons = np.zeros((128, 8), np.float32)
    cons[:, 0:NC_S] = Wb.reshape(NC_S, 128).T               # Wb^T per s-chunk
    cons[:, 4] = bb[0] + bias_b[0]

    in_maps = []
    for core in range(NCORES):
        lo = core * B_LOC
        in_maps.append({
            "qT": np.ascontiguousarray(qT[lo : lo + B_LOC]),
            "kT": kT[lo : lo + B_LOC],
            "wk": wk,
            "bkp": bkp,
            "cons": cons,
        })
    return in_maps


def kernel(q, k, Wk, bk, Wb, bb, bias_b):
    nc = _get_nc()
    in_maps = _prep_inputs(q, k, Wk, bk, Wb, bb, bias_b)
    res = run_bass_kernel_spmd(nc, in_maps, list(range(NCORES)))
    out = np.concatenate([np.asarray(res.results[c]["out"]) for c in range(NCORES)],
                         axis=0)
    return out.astype(np.float32)
